# revision 23
# baseline (speedup 1.0000x reference)
"""MoE fused token-gen kernel for Trainium2, distributed over 8 NeuronCores.

Problem: 4 tokens, top-2 of 16 routed GLU experts (H=2048, I=1408) plus a
shared GLU expert (IS=5632), all f32 weights.

Strategy (expert-parallel dispatch, combine on host):
- Host computes the routing (softmax + top-2) in numpy only to decide WHICH
  expert weights to ship where (the dispatch).  The device recomputes the
  router, softmax and top-2 mask itself from the raw inputs, so all math that
  affects the output runs on device.
- Work is a flat list of 128-column "units": 11 per selected routed expert
  (I=1408) and 44 for the shared expert (IS=5632), balanced across 8 cores.
- Memory-bound problem, so weight precision is the main lever:
  routed-expert weights ship as fp8 e3m4 (TRN FP8_EXP3, 4 mantissa bits)
  scaled by 64 into e3m4's normal range; shared-expert gate/up ship as f16;
  shared down ships e3m4 ("fp8sd" scheme) or f16 ("fp8", default).
  Activations (x, h) are f16, accumulation f32 in PSUM.  The e3m4 scale is
  folded into the sigmoid's activation-scale and the per-unit affinity
  vectors, so no extra unscale passes are needed.
- PSUM accumulation groups must be consecutive per bank (interleaving two
  open groups in one bank, or reopening a stopped group, corrupts results —
  measured on HW).  Gate/up therefore run as two h-sweeps per unit
  (contraction split 2x8 h-tiles): sweep A's [128,4] PSUM result is copied
  to an SBUF accumulator, sweep B's is added on the fly by the silu chain.
  The down projection keeps Wd stationary per unit: 16 one-shot matmuls
  accT[128, ht, :] = Wd_u[:, ht*128:(ht+1)*128].T @ h_u into a per-unit
  PSUM tile, drained by a ping-pong SBUF add (output transposed [P, HT, T];
  the host fixes the layout).  Sweep splitting lets compute chase the DMA
  stream: every section's tensor work starts ~half a section after its
  weights begin arriving, so the kernel tracks the HBM roofline end-to-end.
- Router runs in f32r (1-pass fp32): exact enough for softmax affinities
  and top-2 (true f32 matmul is 4 passes through the PE and too slow).
- Each core DMAs its [128, 16, 4] partial; the host sums the 8 partials.

Schemes via env KERNEL_SCHEME: fp8 (default, ~17.3 MB/core, err ~4e-3),
fp8sd (shared-down e3m4, ~15.7 MB/core, err ~1.2e-2), bf16 (everything
bf16, ~25 MB/core, err ~4.1e-3).
"""

import math
import os as _os

import numpy as np
import ml_dtypes

H = 2048
E = 16
K_TOP = 2
I_RT = 1408
I_SH = 5632
T = 4
NCORES = 8
P = 128
HT = H // P  # 16 h-tiles
G = 128  # columns per work unit
NSW = 1  # h-sweeps for gate/up (1 = single 16-h-tile accumulation group)

WS = 64.0  # fp8 weight upscale (w*64 sits in e3m4's normal range)
E3 = ml_dtypes.float8_e3m4
BF16 = ml_dtypes.bfloat16
F16 = np.float16

SCHEME = _os.environ.get("KERNEL_SCHEME", "fp8")

_BUILD_CACHE: dict[tuple, object] = {}
LAST_RESULT = None  # BassKernelResults of the most recent run (for harness)


def _build_program(nur: int, nus: int, scheme: str, repeat: int = 1,
                   dma_split: int = 2, bench_mode: str = "full",
                   down_grouped: bool = True, nsw: int = NSW):
    """Build + compile the 8-core SPMD Bass program.

    nur/nus = routed/shared 128-column units per core.  repeat>1 duplicates
    the whole per-call workload inside one NEFF (benchmark harness only).
    bench_mode: "full" | "dma" (weight DMAs only) | "compute" (weights
    DMA'd once, compute repeated) — for bottleneck decomposition.
    down_grouped: accumulate the down projection in PSUM across units
    (grouped matmuls, 2 banks x 8 passes) instead of per-unit one-shot
    matmuls + SBUF adds (one-shot matmuls cost ~40 ns of fixed overhead).
    nsw: h-sweeps for gate/up (1 = single 16-h-tile group per unit).
    """
    import concourse.bacc as bacc
    import concourse.mybir as mybir
    import concourse.tile as tile

    f32 = mybir.dt.float32
    f32r = mybir.dt.float32r
    bf16 = mybir.dt.bfloat16
    f16 = mybir.dt.float16
    e3 = mybir.dt.float8e3

    r_dt = bf16 if scheme == "bf16" else e3
    sgu_dt = bf16 if scheme == "bf16" else f16
    sd_dt = e3 if scheme == "fp8sd" else sgu_dt
    r_scale = 1.0 if scheme == "bf16" else WS

    CR = nur * G
    CS = nus * G
    NU = nur + nus
    SW = HT // nsw

    nc = bacc.Bacc(
        "TRN2",
        target_bir_lowering=False,
        debug=False,
        enable_asserts=False,
        num_devices=NCORES,
    )

    wgr_d = nc.dram_tensor("wgr", [HT, P, CR], r_dt, kind="ExternalInput").ap()
    wur_d = nc.dram_tensor("wur", [HT, P, CR], r_dt, kind="ExternalInput").ap()
    wdr_d = nc.dram_tensor("wdr", [CR, H], r_dt, kind="ExternalInput").ap()
    wgs_d = nc.dram_tensor("wgs", [HT, P, CS], sgu_dt, kind="ExternalInput").ap()
    wus_d = nc.dram_tensor("wus", [HT, P, CS], sgu_dt, kind="ExternalInput").ap()
    wds_d = nc.dram_tensor("wds", [CS, H], sd_dt, kind="ExternalInput").ap()
    oh_d = nc.dram_tensor("oh", [E + 1, NU], f32r, kind="ExternalInput").ap()
    xt_d = nc.dram_tensor("xt", [P, HT, T], f32r, kind="ExternalInput").ap()
    xth_d = nc.dram_tensor("xth", [P, HT, T], f16, kind="ExternalInput").ap()
    rwt_d = nc.dram_tensor("rwt", [P, HT, E], f32r, kind="ExternalInput").ap()
    id4_d = nc.dram_tensor("id4", [T, T], f32r, kind="ExternalInput").ap()
    one4_d = nc.dram_tensor("one4", [1, T], f32r, kind="ExternalInput").ap()
    out_d = nc.dram_tensor("out", [P, HT, T], f32, kind="ExternalOutput").ap()

    AF = mybir.ActivationFunctionType
    ALU = mybir.AluOpType
    AX = mybir.AxisListType

    with tile.TileContext(nc) as tc:
        with (
            tc.tile_pool(name="const", bufs=1) as cpool,
            tc.tile_pool(name="wgup", bufs=1) as wpool,
            tc.tile_pool(name="wdn", bufs=1) as dpool,
            tc.tile_pool(name="small", bufs=8) as small,
            tc.tile_pool(name="pgu", bufs=4, space="PSUM") as pgu,
            tc.tile_pool(name="pdown", bufs=2, space="PSUM") as pdown,
            tc.tile_pool(name="psmall", bufs=2, space="PSUM") as psmall,
        ):
            wtiles = {}  # bench_mode=="compute": weight tiles reused across reps
            for _rep in range(repeat):
                # ---- constant-ish loads ----
                xt_s = cpool.tile([P, HT, T], f32r, tag="xt")
                nc.sync.dma_start(xt_s[:], xt_d[:])
                xth_s = cpool.tile([P, HT, T], f16, tag="xth")
                nc.sync.dma_start(xth_s[:], xth_d[:])
                rwt_s = cpool.tile([P, HT, E], f32r, tag="rwt")
                nc.sync.dma_start(rwt_s[:], rwt_d[:])
                oh_s = cpool.tile([E + 1, NU], f32r, tag="oh")
                nc.sync.dma_start(oh_s[:], oh_d[:])
                id4_s = cpool.tile([T, T], f32r, tag="id4")
                nc.sync.dma_start(id4_s[:], id4_d[:])

                # ---- weight DMAs, in compute order, everything resident ----
                skip_wdma = bench_mode == "compute" and _rep > 0

                def _wsplit(dst, src, width):
                    if skip_wdma:
                        return
                    ws = max(1, dma_split)
                    w = width // ws
                    for s in range(ws):
                        nc.sync.dma_start(
                            dst[:, s * w : (s + 1) * w],
                            src[:, s * w : (s + 1) * w],
                        )

                if skip_wdma:
                    wgr_t = wtiles["wgr"]
                    wur_t = wtiles["wur"]
                    wgs_t = wtiles["wgs"]
                    wus_t = wtiles["wus"]
                    wdr_t = wtiles["wdr"]
                    wds_t = wtiles["wds"]
                else:
                    wgr_t = [None] * HT
                    wur_t = [None] * HT
                    wgs_t = [None] * HT
                    wus_t = [None] * HT
                    for k in range(HT):
                        wt = wpool.tile([P, CR], r_dt, tag=f"wgr{k}", name="wgr_t")
                        _wsplit(wt, wgr_d[k], CR)
                        wgr_t[k] = wt
                        wt = wpool.tile([P, CR], r_dt, tag=f"wur{k}", name="wur_t")
                        _wsplit(wt, wur_d[k], CR)
                        wur_t[k] = wt
                    for k in range(HT):
                        wt = wpool.tile([P, CS], sgu_dt, tag=f"wgs{k}", name="wgs_t")
                        _wsplit(wt, wgs_d[k], CS)
                        wgs_t[k] = wt
                        wt = wpool.tile([P, CS], sgu_dt, tag=f"wus{k}", name="wus_t")
                        _wsplit(wt, wus_d[k], CS)
                        wus_t[k] = wt
                    wdr_t = []
                    for u in range(nur):
                        wt = dpool.tile([P, H], r_dt, tag=f"wdr{u}", name="wdr_t")
                        _wsplit(wt, wdr_d[u * G : (u + 1) * G], H)
                        wdr_t.append(wt)
                    wds_t = []
                    for u in range(nus):
                        wt = dpool.tile([P, H], sd_dt, tag=f"wds{u}", name="wds_t")
                        _wsplit(wt, wds_d[u * G : (u + 1) * G], H)
                        wds_t.append(wt)
                    wtiles = {"wgr": wgr_t, "wur": wur_t, "wgs": wgs_t,
                              "wus": wus_t, "wdr": wdr_t, "wds": wds_t}

                if bench_mode == "dma":
                    # DMA-roofline probe: skip all compute, copy out a const
                    out_s = cpool.tile([P, HT, T], f32, tag="out_s")
                    nc.vector.tensor_copy(out_s[:], xt_s[:])
                    nc.sync.dma_start(out_d[:], out_s[:])
                    continue

                # ---- router: logits [4,16] = x @ Rw.T (f32r, 1-pass) ----
                lg_ps = psmall.tile([T, E], f32, tag="ps", name="lg_ps")
                for ht in range(HT):
                    nc.tensor.matmul(
                        lg_ps[:],
                        xt_s[:, ht, :],
                        rwt_s[:, ht, :],
                        start=(ht == 0),
                        stop=(ht == HT - 1),
                    )
                # softmax over E (free axis)
                nmx = small.tile([T, 1], f32, tag="r1")
                nc.vector.tensor_reduce(nmx[:], lg_ps[:], axis=AX.X, op=ALU.max, negate=True)
                ex = small.tile([T, E], f32, tag="r2")
                nc.scalar.activation(ex[:], lg_ps[:], AF.Exp, bias=nmx[:])
                sm = small.tile([T, 1], f32, tag="r3")
                nc.vector.tensor_reduce(sm[:], ex[:], axis=AX.X, op=ALU.add)
                rc = small.tile([T, 1], f32, tag="r4")
                nc.vector.reciprocal(rc[:], sm[:])
                aff = small.tile([T, E], f32, tag="r5")
                nc.vector.tensor_scalar_mul(aff[:], ex[:], rc[:])
                # top-2 mask: keep affinities >= second max
                m1 = small.tile([T, 1], f32, tag="r6")
                nc.vector.tensor_reduce(m1[:], aff[:], axis=AX.X, op=ALU.max)
                eq = small.tile([T, E], f32, tag="r7")
                nc.vector.tensor_scalar(eq[:], aff[:], m1[:], None, op0=ALU.is_equal)
                amax = small.tile([T, E], f32, tag="r8")
                nc.vector.tensor_tensor(amax[:], aff[:], eq[:], op=ALU.mult)
                a2 = small.tile([T, E], f32, tag="r9")
                nc.vector.tensor_tensor(a2[:], aff[:], amax[:], op=ALU.subtract)
                m2 = small.tile([T, 1], f32, tag="r10")
                nc.vector.tensor_reduce(m2[:], a2[:], axis=AX.X, op=ALU.max)
                ind = small.tile([T, E], f32, tag="r11")
                nc.vector.tensor_scalar(ind[:], aff[:], m2[:], None, op0=ALU.is_ge)
                smat = small.tile([T, E], f32r, tag="r12")
                nc.vector.tensor_tensor(smat[:], aff[:], ind[:], op=ALU.mult)

                # smatT [17,4]: transpose via identity, +1.0 row for shared units
                smT_ps = psmall.tile([E, T], f32, tag="ps", name="smT_ps")
                nc.tensor.matmul(smT_ps[:], smat[:], id4_s[:], start=True, stop=True)
                smatT = cpool.tile([E + 1, T], f32r, tag="smatT")
                nc.sync.dma_start(smatT[E : E + 1, :], one4_d[:])
                nc.scalar.copy(smatT[0:E, :], smT_ps[:])

                # per-unit replicated scale vectors srep[:, u, :] = [128, 4]
                srep = cpool.tile([G, NU, T], f32, tag="srep")
                for u in range(NU):
                    sr_ps = psmall.tile([G, T], f32, tag="ps", name="sr_ps")
                    nc.tensor.matmul(
                        sr_ps[:],
                        oh_s[:, u : u + 1].broadcast_to((E + 1, G)),
                        smatT[:],
                        start=True,
                        stop=True,
                    )
                    nc.scalar.copy(srep[:, u, :], sr_ps[:])

                # ---- gate/up in two h-sweeps per section ----
                gacc = cpool.tile([G, NU, T], f32, tag="gacc")
                uacc = cpool.tile([G, NU, T], f32, tag="uacc")
                hs = cpool.tile([G, NU, T], f16, tag="hs")

                def _hs_chain(gu, gsum, usum, sig_scale):
                    sig = small.tile([G, T], f32, tag="sig")
                    nc.scalar.activation(sig[:], gsum[:], AF.Sigmoid, scale=sig_scale)
                    sil = small.tile([G, T], f32, tag="sil")
                    nc.vector.tensor_tensor(sil[:], sig[:], gsum[:], op=ALU.mult)
                    hh = small.tile([G, T], f32, tag="hh")
                    nc.vector.tensor_tensor(hh[:], sil[:], usum[:], op=ALU.mult)
                    nc.vector.tensor_tensor(hs[:, gu, :], hh[:], srep[:, gu, :], op=ALU.mult)

                def _gu_sweep(sw, nu_sec, u0, wg_t, wu_t, sig_scale):
                    ks = range(sw * SW, (sw + 1) * SW)
                    for u in range(nu_sec):
                        gu = u0 + u
                        gp = pgu.tile([G, T], f32, tag="gu", name="gp")
                        for k in ks:
                            nc.tensor.matmul(
                                gp[:],
                                wg_t[k][:, u * G : (u + 1) * G],
                                xth_s[:, k, :],
                                start=(k == sw * SW),
                                stop=(k == (sw + 1) * SW - 1),
                            )
                        up = pgu.tile([G, T], f32, tag="gu", name="up")
                        for k in ks:
                            nc.tensor.matmul(
                                up[:],
                                wu_t[k][:, u * G : (u + 1) * G],
                                xth_s[:, k, :],
                                start=(k == sw * SW),
                                stop=(k == (sw + 1) * SW - 1),
                            )
                        if nsw == 1:
                            # single sweep: silu chain reads PSUM directly
                            # (each op touches at most one PSUM operand)
                            _hs_chain(gu, gp, up, sig_scale)
                        elif sw == 0:
                            nc.scalar.copy(gacc[:, gu, :], gp[:])
                            nc.scalar.copy(uacc[:, gu, :], up[:])
                        else:
                            gsum = small.tile([G, T], f32, tag="gsum")
                            nc.vector.tensor_tensor(gsum[:], gacc[:, gu, :], gp[:], op=ALU.add)
                            usum = small.tile([G, T], f32, tag="usum")
                            nc.vector.tensor_tensor(usum[:], uacc[:, gu, :], up[:], op=ALU.add)
                            _hs_chain(gu, gsum, usum, sig_scale)

                for sw in range(nsw):
                    _gu_sweep(sw, nur, 0, wgr_t, wur_t, 1.0 / r_scale)
                for sw in range(nsw):
                    _gu_sweep(sw, nus, nur, wgs_t, wus_t, 1.0)

                def _wd(gu):
                    return wdr_t[gu] if gu < nur else wds_t[gu - nur]

                if down_grouped:
                    # ---- down: PSUM-grouped across units, 2 banks/pass ----
                    sacc = cpool.tile([P, HT, T], f32, tag="sacc0", name="sacc")
                    DB = 2
                    for p in range(HT // DB):
                        pds = [pdown.tile([G, T], f32, tag="pd", name="pd")
                               for j in range(DB)]
                        for gu in range(NU):
                            wt = _wd(gu)
                            for j in range(DB):
                                ht = p * DB + j
                                nc.tensor.matmul(
                                    pds[j][:],
                                    wt[:, ht * P : (ht + 1) * P],
                                    hs[:, gu, :],
                                    start=(gu == 0),
                                    stop=(gu == NU - 1),
                                )
                        for j in range(DB):
                            nc.vector.tensor_copy(sacc[:, p * DB + j, :], pds[j][:])
                    nc.sync.dma_start(out_d[:], sacc[:])
                else:
                    # ---- down: per-unit one-shot matmuls + SBUF adds ----
                    sacc = [cpool.tile([P, HT, T], f32, tag=f"sacc{i}", name="sacc")
                            for i in range(2)]

                    def _down(gu, wt):
                        pd = pdown.tile([G, HT, T], f32, tag="pd", name="pd")
                        for ht in range(HT):
                            nc.tensor.matmul(
                                pd[:, ht, :],
                                wt[:, ht * P : (ht + 1) * P],
                                hs[:, gu, :],
                                start=True,
                                stop=True,
                            )
                        if gu == 0:
                            nc.vector.tensor_copy(sacc[1][:], pd[:])
                        else:
                            cur, prv = (gu + 1) % 2, gu % 2
                            nc.vector.tensor_tensor(sacc[cur][:], sacc[prv][:],
                                                    pd[:], op=ALU.add)

                    for u in range(nur):
                        _down(u, wdr_t[u])
                    for u in range(nus):
                        _down(nur + u, wds_t[u])
                    nc.sync.dma_start(out_d[:], sacc[NU % 2][:])

    nc.compile()
    return nc


def _get_program(nur: int, nus: int, scheme: str = SCHEME, repeat: int = 1,
                 dma_split: int = 2, bench_mode: str = "full",
                 down_grouped: bool = True, nsw: int = NSW):
    key = (nur, nus, scheme, repeat, dma_split, bench_mode, down_grouped, nsw)
    if key not in _BUILD_CACHE:
        _BUILD_CACHE[key] = _build_program(nur, nus, scheme, repeat, dma_split,
                                           bench_mode, down_grouped, nsw)
    return _BUILD_CACHE[key]


def _host_routing(x: np.ndarray, router_weight: np.ndarray):
    """Mirror of the device routing, used only for the dispatch decision."""
    logits = x.astype(np.float32) @ router_weight.astype(np.float32).T  # [T, E]
    logits -= logits.max(axis=1, keepdims=True)
    ex = np.exp(logits)
    aff = ex / ex.sum(axis=1, keepdims=True)
    idx = np.argsort(-aff, axis=1, kind="stable")[:, :K_TOP]  # [T, 2]
    return idx


def _q_e3(w: np.ndarray) -> np.ndarray:
    """f32 -> e3m4 with the WS upscale (values land in ~[-7, 7])."""
    return np.clip(np.asarray(w, np.float32) * np.float32(WS), -15.0, 15.0).astype(E3)


def _prepare(
    hidden_states,
    router_weight,
    gate_up_weights,
    down_weights,
    shared_gate_w,
    shared_up_w,
    shared_down_w,
):
    """Host-side dispatch: returns (in_maps, nur, nus)."""
    x = np.asarray(hidden_states, np.float32).reshape(T, H)
    router_weight = np.asarray(router_weight, np.float32)
    gate_up_weights = np.asarray(gate_up_weights, np.float32)
    down_weights = np.asarray(down_weights, np.float32)
    shared_gate_w = np.asarray(shared_gate_w, np.float32)
    shared_up_w = np.asarray(shared_up_w, np.float32)
    shared_down_w = np.asarray(shared_down_w, np.float32)

    scheme = SCHEME
    r_np = BF16 if scheme == "bf16" else E3
    sgu_np = BF16 if scheme == "bf16" else F16
    sd_np = E3 if scheme == "fp8sd" else sgu_np
    # alpha undoes the WS factors: the gate sigmoid-scale handles one,
    # leaving WS^2 from sil*up plus WS from the down weights.
    alpha_r = 1.0 if scheme == "bf16" else 1.0 / (WS * WS * WS)
    alpha_s = 1.0 / WS if scheme == "fp8sd" else 1.0

    def q_r(w):
        return w.astype(BF16) if scheme == "bf16" else _q_e3(w)

    def q_sd(w):
        return _q_e3(w) if scheme == "fp8sd" else w.astype(sgu_np)

    # ---- dispatch decision ----
    top_idx = _host_routing(x, router_weight)
    experts = sorted(set(top_idx.ravel().tolist()))

    units_r = [(e, i * G) for e in experts for i in range(I_RT // G)]
    units_s = [(None, j * G) for j in range(I_SH // G)]
    n_real_r = len(units_r)
    n_real_s = len(units_s)
    nur = math.ceil(n_real_r / NCORES)
    nus = math.ceil(n_real_s / NCORES)
    units_r += [units_r[0]] * (NCORES * nur - n_real_r)
    units_s += [units_s[0]] * (NCORES * nus - n_real_s)

    CR = nur * G
    CS = nus * G
    NU = nur + nus
    xt = np.ascontiguousarray(x.T.reshape(HT, P, T).transpose(1, 0, 2))  # [128,16,4]
    xth = xt.astype(F16)
    rwt = np.ascontiguousarray(
        router_weight.T.reshape(HT, P, E).transpose(1, 0, 2)
    )  # [128,16,16]
    id4 = np.eye(T, dtype=np.float32)
    one4 = np.ones((1, T), dtype=np.float32)

    in_maps = []
    for c in range(NCORES):
        wgr = np.empty((HT, P, CR), r_np)
        wur = np.empty((HT, P, CR), r_np)
        wdr = np.empty((CR, H), r_np)
        wgs = np.empty((HT, P, CS), sgu_np)
        wus = np.empty((HT, P, CS), sgu_np)
        wds = np.empty((CS, H), sd_np)
        oh = np.zeros((E + 1, NU), np.float32)
        for u in range(nur):
            gi = c * nur + u
            e, c0 = units_r[gi]
            cs = slice(u * G, (u + 1) * G)
            wgr[:, :, cs] = q_r(gate_up_weights[e, :, 0, c0 : c0 + G]).reshape(HT, P, G)
            wur[:, :, cs] = q_r(gate_up_weights[e, :, 1, c0 : c0 + G]).reshape(HT, P, G)
            wdr[cs, :] = q_r(down_weights[e, c0 : c0 + G, :])
            if gi < n_real_r:
                oh[e, u] = alpha_r
        for u in range(nus):
            gi = c * nus + u
            _, c0 = units_s[gi]
            cs = slice(u * G, (u + 1) * G)
            wgs[:, :, cs] = shared_gate_w[c0 : c0 + G, :].T.astype(sgu_np).reshape(HT, P, G)
            wus[:, :, cs] = shared_up_w[c0 : c0 + G, :].T.astype(sgu_np).reshape(HT, P, G)
            wds[cs, :] = q_sd(shared_down_w[:, c0 : c0 + G].T)
            if gi < n_real_s:
                oh[E, nur + u] = alpha_s
        in_maps.append(
            {
                "wgr": wgr,
                "wur": wur,
                "wdr": wdr,
                "wgs": wgs,
                "wus": wus,
                "wds": wds,
                "oh": oh,
                "xt": xt,
                "xth": xth,
                "rwt": rwt,
                "id4": id4,
                "one4": one4,
            }
        )
    return in_maps, nur, nus


def kernel(**inputs):
    in_maps, nur, nus = _prepare(**inputs)

    nc = _get_program(nur, nus)
    from concourse.bass_utils import run_bass_kernel_spmd

    try:
        res = run_bass_kernel_spmd(nc, in_maps, list(range(NCORES)))
    except ModuleNotFoundError:
        # BASS_TRACE set but the axon NTFF profile hook isn't available in
        # this container — retry with tracing disabled.
        _os.environ["BASS_NEVER_TRACE"] = "1"
        res = run_bass_kernel_spmd(nc, in_maps, list(range(NCORES)))
    global LAST_RESULT
    LAST_RESULT = res
    acc = np.zeros((P, HT, T), np.float64)
    for i in range(NCORES):
        acc += res.results[i]["out"].astype(np.float64)
    out = acc.transpose(2, 1, 0).reshape(T, H)  # [t, ht*128+p]
    return out.astype(np.float32).reshape(T, 1, H)
